# Initial kernel scaffold
#
"""Trainium2 Bass kernel for nn_InterpolationModel (NaN-gap linear interpolation).

Problem: x [256, 2048, 22, 2] f32, one contiguous NaN gap along T per batch row.
Output: x with the gap filled by linear interpolation between the last valid
frame before the gap (s) and the first valid frame after it (e).

Strategy (pure data parallel over batch, 32 rows per core):
  - Bulk copy x -> y through SBUF in 4 chunks of 8 rows ([128, 5632] tiles,
    partition = 128 consecutive frames, 22.5KB contiguous per partition).
  - While each chunk transits SBUF, sample element 0 of every frame
    (stride-44 AP) and reduce per partition: min(t + 65536*valid) and
    min((valid-1)*t) -> first/last NaN frame per partition.
  - One TensorE transpose + grouped reduce turns per-partition partials into
    per-row s (last valid before gap), e (first valid after), 1/(e-s).
  - Per-row scalars round-trip through a tiny DRAM scratch so an indirect
    gather can replicate them to a [128, .] layout (4 partitions per row).
  - Fixed 512-frame window starting at s+1 always covers the whole gap
    (gap <= 511) and never leaves the row (s < 1024 => s+513 <= 1536 < 2048).
    Gather the window [128, 5632], compute interp = xs + (t-s)*slope with
    broadcast APs, keep original values where not NaN (copy_predicated),
    scatter back over y at the same offsets.

Everything is a single Tile-scheduled program; the only cross-phase sync
needed beyond Tile's tracking is "scatter after bulk stores" and
"scalar gather after scratch write", wired with add_dep_helper.
"""

import numpy as np

import concourse.bacc as bacc
import concourse.bass as bass
import concourse.mybir as mybir
import concourse.tile as tile
from bass_rust import add_dep_helper
from concourse.masks import make_identity

F32 = mybir.dt.float32
I32 = mybir.dt.int32
AX = mybir.AxisListType
OP = mybir.AluOpType

# Full problem: B=256, T=2048, A=22, D=2 over 8 cores.
B, T, A, D = 256, 2048, 22, 2
C = A * D            # 44 contiguous f32 per frame
NCORES = 8
R = B // NCORES      # 32 rows per core
CHUNKS = 4           # bulk-copy chunks per core
RCH = R // CHUNKS    # 8 rows per chunk
P = 128
FPP = (RCH * T) // P  # 128 frames per partition in a chunk tile
PPR = T // FPP        # 16 partitions per row
WSUB = 4              # window partitions per row (4 * 32 rows = 128)
WF = 128              # frames per window partition (window = 512 frames)
BIG = 65536.0


def _bcast_mid(ap, count):
    """[P, n] AP -> [P, count, n] with a stride-0 middle axis."""
    return bass.AP(ap.tensor, ap.offset, [list(ap.ap[0]), [0, count], list(ap.ap[1])])


def _bcast_inner(ap, count):
    """[P, n] AP -> [P, n, count] with a stride-0 inner axis."""
    return bass.AP(ap.tensor, ap.offset, [list(ap.ap[0]), list(ap.ap[1]), [0, count]])


def _ins(bi):
    return bi.ins if hasattr(bi, "ins") else bi


def build_kernel(tc, x, y):
    nc = tc.nc
    xv = x.rearrange("b t c -> (b t) c")   # [R*T, C] frame rows, offset 0
    yv = y.rearrange("b t c -> (b t) c")

    from contextlib import ExitStack

    with ExitStack() as ctx:
        const = ctx.enter_context(tc.tile_pool(name="const", bufs=1))
        data = ctx.enter_context(tc.tile_pool(name="data", bufs=3))
        det = ctx.enter_context(tc.tile_pool(name="det", bufs=2))
        small = ctx.enter_context(tc.tile_pool(name="small", bufs=1))
        win = ctx.enter_context(tc.tile_pool(name="win", bufs=1))
        psum = ctx.enter_context(tc.tile_pool(name="psum", bufs=1, space="PSUM"))
        dram = ctx.enter_context(tc.tile_pool(name="dram", bufs=1, space="DRAM"))
        # ---- constants (built on device) ----
        ident = const.tile([P, P], F32)
        make_identity(nc, ident[:])
        # I128f[p, j] = 128*p + j  (= row_in_chunk*2048 + t_in_row)
        i128f = const.tile([P, FPP], F32)
        nc.gpsimd.iota(i128f[:], pattern=[[1, FPP]], base=0,
                       channel_multiplier=FPP,
                       allow_small_or_imprecise_dtypes=True)
        # Fgrid[p, f] = f
        fgrid = const.tile([P, WF], F32)
        nc.gpsimd.iota(fgrid[:], pattern=[[1, WF]], base=0,
                       channel_multiplier=0,
                       allow_small_or_imprecise_dtypes=True)
        # C2048[c, i] = 2048 * i   (row-in-chunk offset correction)
        c2048 = const.tile([CHUNKS, RCH], F32)
        nc.gpsimd.iota(c2048[:], pattern=[[T, RCH]], base=0,
                       channel_multiplier=0,
                       allow_small_or_imprecise_dtypes=True)
        # per-partition helpers
        pidx = const.tile([P, 1], I32)
        nc.gpsimd.iota(pidx[:], pattern=[[1, 1]], base=0, channel_multiplier=1)
        idx4 = const.tile([P, 1], I32)          # p // 4  (scratch gather idx)
        nc.vector.tensor_scalar(out=idx4[:], in0=pidx[:], scalar1=2,
                                scalar2=None, op0=OP.arith_shift_right)
        d128i = const.tile([P, 1], I32)
        nc.vector.tensor_scalar(out=d128i[:], in0=idx4[:], scalar1=11,
                                scalar2=None, op0=OP.arith_shift_left)
        d128f = const.tile([P, 1], F32)         # (p//4) * 2048
        nc.vector.tensor_copy(out=d128f[:], in_=d128i[:])
        pm4 = const.tile([P, 1], I32)
        nc.vector.tensor_scalar(out=pm4[:], in0=pidx[:], scalar1=3,
                                scalar2=None, op0=OP.bitwise_and)
        w128i = const.tile([P, 1], I32)
        nc.vector.tensor_scalar(out=w128i[:], in0=pm4[:], scalar1=128, scalar2=1,
                                op0=OP.mult, op1=OP.add)
        w128f = const.tile([P, 1], F32)         # 1 + 128*(p%4)
        nc.vector.tensor_copy(out=w128f[:], in_=w128i[:])

        # ---- bulk copy + per-partition gap detection ----
        m = small.tile([P, 2 * CHUNKS], F32)    # cols 0-3 min-partials, 4-7 neg-max
        stores = []
        for k in range(CHUNKS):
            xk = data.tile([P, FPP * C], F32)
            src = x[k * RCH:(k + 1) * RCH].rearrange(
                "r (q j) c -> (r q) (j c)", q=PPR)
            nc.sync.dma_start(out=xk[:], in_=src)

            samp = xk[:].rearrange("p (j c) -> p j c", c=C)[:, :, 0:1]
            v = det.tile([P, FPP], F32)
            nc.vector.tensor_tensor(
                out=v[:].rearrange("p (j o) -> p j o", o=1),
                in0=samp, in1=samp, op=OP.is_equal)
            bb = det.tile([P, FPP], F32)
            # valid*BIG + t_chunk : min over j = first NaN t (per partition)
            nc.vector.scalar_tensor_tensor(
                out=bb[:], in0=v[:], scalar=BIG, in1=i128f[:],
                op0=OP.mult, op1=OP.add)
            nc.vector.tensor_reduce(out=m[:, k:k + 1], in_=bb[:],
                                    axis=AX.X, op=OP.min)
            dd = det.tile([P, FPP], F32)
            # (valid-1)*t_chunk = -(t on NaN frames) : min over j = -last NaN t
            nc.vector.scalar_tensor_tensor(
                out=dd[:], in0=v[:], scalar=-1.0, in1=i128f[:],
                op0=OP.add, op1=OP.mult)
            nc.vector.tensor_reduce(out=m[:, CHUNKS + k:CHUNKS + k + 1],
                                    in_=dd[:], axis=AX.X, op=OP.min)

            dst = y[k * RCH:(k + 1) * RCH].rearrange(
                "r (q j) c -> (r q) (j c)", q=PPR)
            st = nc.sync.dma_start(out=dst, in_=xk[:])
            stores.append(st)

        # ---- cross-partition reduce to per-row s, e, 1/(e-s) ----
        mt = psum.tile([2 * CHUNKS, P], F32)
        nc.tensor.transpose(out=mt[:], in_=m[:], identity=ident[:])
        mins = small.tile([2 * CHUNKS, RCH], F32)
        nc.vector.tensor_reduce(
            out=mins[:], in_=mt[:].rearrange("p (i w) -> p i w", w=PPR),
            axis=AX.X, op=OP.min)
        # rows 0-3: first_nan + 2048*i ; rows 4-7: -(last_nan + 2048*i)
        n2 = small.tile([CHUNKS, RCH], F32)
        nc.sync.dma_start(out=n2[:], in_=mins[CHUNKS:2 * CHUNKS, :])

        fn4 = mins[0:CHUNKS, :]
        pk = small.tile([CHUNKS, RCH * 4], F32)
        nc.vector.memset(pk[:], 0.0)
        pkv = pk[:].rearrange("c (i k) -> c i k", k=4)
        c2v = c2048[:].rearrange("c (i o) -> c i o", o=1)
        # s = first_nan - 2048*i - 1
        nc.vector.scalar_tensor_tensor(
            out=pkv[:, :, 0:1],
            in0=fn4.rearrange("c (i o) -> c i o", o=1), scalar=-1.0,
            in1=c2v, op0=OP.add, op1=OP.subtract)
        # e = last_nan + 1 = -(n2 + 2048*i) + 1
        t2 = small.tile([CHUNKS, RCH], F32)
        nc.vector.tensor_tensor(
            out=t2[:].rearrange("c (i o) -> c i o", o=1),
            in0=n2[:].rearrange("c (i o) -> c i o", o=1),
            in1=c2v, op=OP.add)
        nc.vector.tensor_scalar(
            out=pkv[:, :, 1:2],
            in0=t2[:].rearrange("c (i o) -> c i o", o=1),
            scalar1=-1.0, scalar2=1.0, op0=OP.mult, op1=OP.add)
        # 1 / (e - s)
        es = small.tile([CHUNKS, RCH], F32)
        nc.vector.tensor_tensor(
            out=es[:].rearrange("c (i o) -> c i o", o=1),
            in0=pkv[:, :, 1:2], in1=pkv[:, :, 0:1], op=OP.subtract)
        nc.vector.reciprocal(
            out=pkv[:, :, 2:3],
            in_=es[:].rearrange("c (i o) -> c i o", o=1))

        scr = dram.tile([R, 4], F32)
        wsc = nc.sync.dma_start(
            out=scr[:].rearrange("(c i) k -> c (i k)", c=CHUNKS), in_=pk[:])

        # ---- replicate per-row scalars to [128, .] via indirect gather ----
        g = small.tile([P, 4], F32)
        gi = nc.gpsimd.indirect_dma_start(
            out=g[:], out_offset=None, in_=scr[:],
            in_offset=bass.IndirectOffsetOnAxis(ap=idx4[:, 0:1], axis=0))
        add_dep_helper(_ins(gi), _ins(wsc), reason="gather scalars after scratch write")

        fxs = small.tile([P, 1], F32)
        nc.vector.tensor_tensor(out=fxs[:], in0=g[:, 0:1], in1=d128f[:], op=OP.add)
        ixs = small.tile([P, 1], I32)
        nc.vector.tensor_copy(out=ixs[:], in_=fxs[:])
        fxe = small.tile([P, 1], F32)
        nc.vector.tensor_tensor(out=fxe[:], in0=g[:, 1:2], in1=d128f[:], op=OP.add)
        ixe = small.tile([P, 1], I32)
        nc.vector.tensor_copy(out=ixe[:], in_=fxe[:])
        fww = small.tile([P, 1], F32)
        nc.vector.tensor_tensor(out=fww[:], in0=fxs[:], in1=w128f[:], op=OP.add)
        ixw = small.tile([P, 1], I32)
        nc.vector.tensor_copy(out=ixw[:], in_=fww[:])

        xs = small.tile([P, C], F32)
        nc.gpsimd.indirect_dma_start(
            out=xs[:], out_offset=None, in_=xv,
            in_offset=bass.IndirectOffsetOnAxis(ap=ixs[:, 0:1], axis=0))
        xe = small.tile([P, C], F32)
        nc.gpsimd.indirect_dma_start(
            out=xe[:], out_offset=None, in_=xv,
            in_offset=bass.IndirectOffsetOnAxis(ap=ixe[:, 0:1], axis=0))

        df = small.tile([P, C], F32)
        nc.vector.tensor_tensor(out=df[:], in0=xe[:], in1=xs[:], op=OP.subtract)
        slope = small.tile([P, C], F32)
        nc.vector.tensor_scalar(out=slope[:], in0=df[:], scalar1=g[:, 2:3],
                                scalar2=None, op0=OP.mult)
        base = small.tile([P, C], F32)
        # base = xs + (1 + 128*(p%4)) * slope
        nc.vector.scalar_tensor_tensor(
            out=base[:], in0=slope[:], scalar=w128f[:, 0:1], in1=xs[:],
            op0=OP.mult, op1=OP.add)

        # ---- window gather, interp, select, scatter ----
        xw = win.tile([P, WF * C], F32)
        nc.gpsimd.indirect_dma_start(
            out=xw[:], out_offset=None, in_=xv,
            in_offset=bass.IndirectOffsetOnAxis(ap=ixw[:, 0:1], axis=0))

        prod = win.tile([P, WF * C], F32)
        prod3 = prod[:].rearrange("p (f c) -> p f c", c=C)
        nc.vector.tensor_tensor(
            out=prod3, in0=_bcast_inner(fgrid[:], C),
            in1=_bcast_mid(slope[:], WF), op=OP.mult)
        nc.vector.tensor_tensor(
            out=prod3, in0=prod3, in1=_bcast_mid(base[:], WF), op=OP.add)
        vm = win.tile([P, WF * C], mybir.dt.uint8)
        nc.vector.tensor_tensor(out=vm[:], in0=xw[:], in1=xw[:], op=OP.is_equal)
        nc.vector.copy_predicated(out=prod[:], mask=vm[:], data=xw[:])

        sc = nc.gpsimd.indirect_dma_start(
            out=yv, out_offset=bass.IndirectOffsetOnAxis(ap=ixw[:, 0:1], axis=0),
            in_=prod[:], in_offset=None)
        for st in stores:
            add_dep_helper(_ins(sc), _ins(st), reason="scatter windows after bulk store")


_NC = None


def _get_nc():
    global _NC
    if _NC is None:
        nc = bacc.Bacc("TRN2", target_bir_lowering=False, debug=False,
                       num_devices=NCORES)
        x = nc.dram_tensor("x", [R, T, C], F32, kind="ExternalInput")
        y = nc.dram_tensor("y", [R, T, C], F32, kind="ExternalOutput")
        with tile.TileContext(nc) as tc:
            build_kernel(tc, x.ap(), y.ap())
        nc.compile()
        _NC = nc
    return _NC


def kernel(x):
    from concourse.bass_utils import run_bass_kernel_spmd

    x = np.ascontiguousarray(x, dtype=np.float32)
    assert x.shape == (B, T, A, D), x.shape
    xr = x.reshape(NCORES, R, T, C)
    nc = _get_nc()
    in_maps = [{"x": xr[i]} for i in range(NCORES)]
    res = run_bass_kernel_spmd(nc, in_maps, core_ids=list(range(NCORES)))
    out = np.stack([res.results[i]["y"] for i in range(NCORES)])
    return out.reshape(B, T, A, D)



# revision 26
# speedup vs baseline: 2.7622x; 2.7622x over previous
"""Trainium2 Bass kernel for nn_InterpolationModel (NaN-gap linear interpolation).

Problem: x [256, 2048, 22, 2] f32, one contiguous NaN gap along T per batch row.
Output: x with the gap filled by linear interpolation between the last valid
frame before the gap (s) and the first valid frame after it (e).

Strategy (pure data parallel over batch, 32 rows per core):
  - Bulk copy x -> SBUF -> y in 4 chunks of 8 rows ([128, 5632] tiles,
    partition (r, q) = 128 consecutive frames of row r, 22.5 KB contiguous
    per partition; 128-partition DMAs spread across all 16 DMA engines).
    All 4 loads are emitted before the stores so reads stream first; stores
    never wait on patching (they copy the NaN gap as-is).
  - While chunks transit SBUF: per-partition NaN-frame count n_p and
    frame-index sum ts_p (element 0 of each frame; the gap blanks whole
    frames). One accumulating TensorE matmul per chunk with a 0/1
    row-membership matrix reduces (n_p, ts_p) into a single [32, 2] PSUM
    tile across all chunks. The gap is contiguous, so
    first_nan = ts/n - (n-1)/2 (rounded to the exact integer with a
    trunc/round-agnostic fixup); s = first_nan - 1, e = first_nan + n,
    slope = (xe - xs)/(n+1) with xs/xe fetched by 32-descriptor indirect
    gathers.
  - Patch via a fixed 512-frame window per row starting at s+1 (s < 1024,
    gap <= 511, so the window always covers the gap and stays in-row):
    gather windows [128, 5632] from x (4 partitions per row), interp =
    xs + w * slope with w = 1 + 128*(p%4) + j static per partition, copy
    interp over the NaN elements only (uint8 mask), scatter the windows
    back over y after the bulk stores.
"""

import numpy as np

import concourse.bacc as bacc
import concourse.bass as bass
import concourse.mybir as mybir
import concourse.tile as tile
from bass_rust import add_dep_helper

F32 = mybir.dt.float32
I32 = mybir.dt.int32
U8 = mybir.dt.uint8
AX = mybir.AxisListType
OP = mybir.AluOpType

# Full problem: B=256, T=2048, A=22, D=2 over 8 cores.
B, T, A, D = 256, 2048, 22, 2
C = A * D            # 44 contiguous f32 per frame
NCORES = 8
R = B // NCORES      # 32 rows per core
P = 128

FPP = 128            # frames per partition
QPR = T // FPP       # 16 partitions per row
CHUNKS = 4           # bulk chunks per core
RCH = R // CHUNKS    # 8 rows per chunk
WPR = 4              # window partitions per row (4*128 = 512-frame window)
NB = 4 + 2 * C       # row record: s, inv, scratch, win_start, xs[44], slope[44]


def _bcast_mid(ap, count):
    """[P, n] AP -> [P, count, n] with a stride-0 middle axis."""
    return bass.AP(ap.tensor, ap.offset, [list(ap.ap[0]), [0, count], list(ap.ap[1])])


def _bcast_inner(ap, count):
    """[P, n] AP -> [P, n, count] with a stride-0 inner axis."""
    return bass.AP(ap.tensor, ap.offset, [list(ap.ap[0]), list(ap.ap[1]), [0, count]])


def _ins(bi):
    return bi.ins if hasattr(bi, "ins") else bi


def build_kernel(tc, x, y):
    nc = tc.nc
    xv = x.rearrange("b t c -> (b t) c")   # [R*T, C] frame rows, offset 0
    yv = y.rearrange("b t c -> (b t) c")

    from contextlib import ExitStack

    with ExitStack() as ctx:
        const = ctx.enter_context(tc.tile_pool(name="const", bufs=1))
        cpsum = ctx.enter_context(tc.tile_pool(name="cpsum", bufs=1, space="PSUM"))
        data = ctx.enter_context(tc.tile_pool(name="data", bufs=CHUNKS))
        det = ctx.enter_context(tc.tile_pool(name="det", bufs=2))
        sm = ctx.enter_context(tc.tile_pool(name="sm", bufs=1))
        win = ctx.enter_context(tc.tile_pool(name="win", bufs=1))
        psum = ctx.enter_context(tc.tile_pool(name="psum", bufs=1, space="PSUM"))

        # ---- constants ----
        # Gk[k][p, j] = (j == k*RCH + p // QPR): per-chunk row-membership
        # matrices mapping chunk partitions to the global [32]-row space.
        Gk = []
        for k in range(CHUNKS):
            g = const.tile([P, R], F32, name=f"Gk{k}")
            nc.gpsimd.memset(g[:], 1.0)
            nc.gpsimd.affine_select(out=g[:], in_=g[:], compare_op=OP.is_ge,
                                    fill=0.0, base=P * k,
                                    pattern=[[-QPR, R]], channel_multiplier=1)
            nc.gpsimd.affine_select(out=g[:], in_=g[:], compare_op=OP.is_ge,
                                    fill=0.0, base=QPR - 1 - P * k,
                                    pattern=[[QPR, R]], channel_multiplier=-1)
            Gk.append(g)
        # Bm[r, p] = (p // QPR == r): used to build i128q below
        Bm = const.tile([RCH, P], F32)
        nc.gpsimd.memset(Bm[:], 1.0)
        nc.gpsimd.affine_select(out=Bm[:], in_=Bm[:], compare_op=OP.is_ge,
                                fill=0.0, base=0, pattern=[[1, P]],
                                channel_multiplier=-QPR)
        nc.gpsimd.affine_select(out=Bm[:], in_=Bm[:], compare_op=OP.is_ge,
                                fill=0.0, base=QPR - 1, pattern=[[-1, P]],
                                channel_multiplier=QPR)
        # Bw[v, wp] = (wp // WPR == v): window broadcast matrix
        Bw = const.tile([R, P], F32)
        nc.gpsimd.memset(Bw[:], 1.0)
        nc.gpsimd.affine_select(out=Bw[:], in_=Bw[:], compare_op=OP.is_ge,
                                fill=0.0, base=0, pattern=[[-1, R], [0, WPR]],
                                channel_multiplier=1)
        nc.gpsimd.affine_select(out=Bw[:], in_=Bw[:], compare_op=OP.is_ge,
                                fill=0.0, base=0, pattern=[[1, R], [0, WPR]],
                                channel_multiplier=-1)
        rowri = const.tile([RCH, 1], I32)
        nc.gpsimd.iota(rowri[:], pattern=[[1, 1]], base=0, channel_multiplier=1)
        rowrf = const.tile([RCH, 1], F32)
        nc.vector.tensor_copy(out=rowrf[:], in_=rowri[:])
        # rowoff32[r] = r * T for the 32 per-core rows
        rowo32i = const.tile([R, 1], I32)
        nc.gpsimd.iota(rowo32i[:], pattern=[[1, 1]], base=0,
                       channel_multiplier=T)
        rowoff32 = const.tile([R, 1], F32)
        nc.vector.tensor_copy(out=rowoff32[:], in_=rowo32i[:])
        # i128q[p, j] = (p % QPR) * FPP + j  (frame index within the row)
        i128q = const.tile([P, FPP], F32)
        nc.gpsimd.iota(i128q[:], pattern=[[1, FPP]], base=0,
                       channel_multiplier=FPP,
                       allow_small_or_imprecise_dtypes=True)
        rb = cpsum.tile([P, 1], F32)
        nc.tensor.matmul(rb[:], lhsT=Bm[:], rhs=rowrf[:], start=True, stop=True)
        rbtg = const.tile([P, 1], F32)
        nc.vector.tensor_scalar(out=rbtg[:], in0=rb[:], scalar1=float(T),
                                scalar2=None, op0=OP.mult)
        nc.vector.tensor_scalar(out=i128q[:], in0=i128q[:],
                                scalar1=rbtg[:, 0:1], scalar2=None,
                                op0=OP.subtract)
        # window frame offsets: woff[wp] = 128 * (wp % 4), wgrid = 1 + woff + j
        pidx = const.tile([P, 1], I32)
        nc.gpsimd.iota(pidx[:], pattern=[[1, 1]], base=0, channel_multiplier=1)
        pm4 = const.tile([P, 1], I32)
        nc.vector.tensor_scalar(out=pm4[:], in0=pidx[:], scalar1=WPR - 1,
                                scalar2=None, op0=OP.bitwise_and)
        w128i = const.tile([P, 1], I32)
        nc.vector.tensor_scalar(out=w128i[:], in0=pm4[:], scalar1=7,
                                scalar2=None, op0=OP.arith_shift_left)
        wofff = const.tile([P, 1], F32)
        nc.vector.tensor_copy(out=wofff[:], in_=w128i[:])
        wgrid = const.tile([P, FPP], F32)
        nc.gpsimd.iota(wgrid[:], pattern=[[1, FPP]], base=0,
                       channel_multiplier=0,
                       allow_small_or_imprecise_dtypes=True)
        nc.vector.tensor_scalar(out=wgrid[:], in0=wgrid[:],
                                scalar1=wofff[:, 0:1], scalar2=1.0,
                                op0=OP.add, op1=OP.add)

        # ---- bulk loads + per-chunk detection (loads emitted before stores
        # so the DMA ring streams all reads first) ----
        nts = psum.tile([R, 2], F32)
        xks = []
        for k in range(CHUNKS):
            xk = data.tile([P, FPP * C], F32)
            src = x[k * RCH:(k + 1) * RCH].rearrange(
                "r (q j) c -> (r q) (j c)", q=QPR)
            nc.sync.dma_start(out=xk[:], in_=src)
            xks.append(xk)
            xk3 = xk[:].rearrange("p (j c) -> p j c", c=C)

            samp = xk3[:, :, 0:1]
            nanf = det.tile([P, FPP], F32)
            nc.vector.tensor_tensor(
                out=nanf[:].rearrange("p (j o) -> p j o", o=1),
                in0=samp, in1=samp, op=OP.not_equal)
            pt = det.tile([P, 2], F32)
            wt = det.tile([P, FPP], F32)
            nc.vector.scalar_tensor_tensor(
                out=wt[:], in0=nanf[:], scalar=1.0, in1=i128q[:],
                op0=OP.mult, op1=OP.mult, accum_out=pt[:, 1:2])
            nc.vector.tensor_reduce(out=pt[:, 0:1], in_=nanf[:],
                                    axis=AX.X, op=OP.add)
            nc.tensor.matmul(nts[:], lhsT=Gk[k][:], rhs=pt[:],
                             start=(k == 0), stop=(k == CHUNKS - 1))

        stores = []
        for k in range(CHUNKS):
            dst = y[k * RCH:(k + 1) * RCH].rearrange(
                "r (q j) c -> (r q) (j c)", q=QPR)
            st = nc.sync.dma_start(out=dst, in_=xks[k][:])
            stores.append(st)

        # ---- row math for all 32 rows at once ----
        vals = sm.tile([R, NB], F32)
        invn = sm.tile([R, 1], F32)
        nc.vector.reciprocal(out=invn[:], in_=nts[:, 0:1])
        mean = sm.tile([R, 1], F32)
        nc.vector.tensor_tensor(out=mean[:], in0=nts[:, 1:2], in1=invn[:],
                                op=OP.mult)
        # f ~= first_nan (+-0.01): mean - 0.5*n + 0.5
        f = sm.tile([R, 1], F32)
        nc.vector.scalar_tensor_tensor(out=f[:], in0=nts[:, 0:1], scalar=-0.5,
                                       in1=mean[:], op0=OP.mult, op1=OP.add)
        nc.vector.tensor_scalar(out=f[:], in0=f[:], scalar1=0.5,
                                scalar2=None, op0=OP.add)
        # round to the exact integer, correct under either trunc or rne convert
        fi = sm.tile([R, 1], I32)
        nc.vector.tensor_copy(out=fi[:], in_=f[:])
        f2 = sm.tile([R, 1], F32)
        nc.vector.tensor_copy(out=f2[:], in_=fi[:])
        dd = sm.tile([R, 1], F32)
        nc.vector.tensor_tensor(out=dd[:], in0=f[:], in1=f2[:], op=OP.subtract)
        bb = sm.tile([R, 1], F32)
        nc.vector.tensor_scalar(out=bb[:], in0=dd[:], scalar1=0.5,
                                scalar2=None, op0=OP.is_gt)
        first = sm.tile([R, 1], F32)
        nc.vector.tensor_tensor(out=first[:], in0=f2[:], in1=bb[:], op=OP.add)
        # vals: col0 = s, col1 = inv = 1/(n+1), col2 = n+1,
        # col3 = global window start
        nc.vector.tensor_scalar(out=vals[:, 0:1], in0=first[:], scalar1=-1.0,
                                scalar2=None, op0=OP.add)
        nc.vector.tensor_scalar(out=vals[:, 2:3], in0=nts[:, 0:1], scalar1=1.0,
                                scalar2=None, op0=OP.add)
        nc.vector.reciprocal(out=vals[:, 1:2], in_=vals[:, 2:3])
        nc.vector.tensor_tensor(out=vals[:, 3:4], in0=first[:],
                                in1=rowoff32[:], op=OP.add)
        # gather indices: xs at first-1+r*T, xe at first+n+r*T
        gi = sm.tile([R, 2], F32)
        nc.vector.tensor_scalar(out=gi[:, 0:1], in0=vals[:, 3:4], scalar1=-1.0,
                                scalar2=None, op0=OP.add)
        nc.vector.scalar_tensor_tensor(out=gi[:, 1:2], in0=nts[:, 0:1],
                                       scalar=1.0, in1=gi[:, 0:1],
                                       op0=OP.add, op1=OP.add)
        gii = sm.tile([R, 2], I32)
        nc.vector.tensor_copy(out=gii[:], in_=gi[:])
        xe32 = sm.tile([R, C], F32)
        nc.gpsimd.indirect_dma_start(
            out=vals[:, 4:4 + C], out_offset=None, in_=xv,
            in_offset=bass.IndirectOffsetOnAxis(ap=gii[:, 0:1], axis=0))
        nc.gpsimd.indirect_dma_start(
            out=xe32[:], out_offset=None, in_=xv,
            in_offset=bass.IndirectOffsetOnAxis(ap=gii[:, 1:2], axis=0))
        dsl = sm.tile([R, C], F32)
        nc.vector.tensor_tensor(out=dsl[:], in0=xe32[:], in1=vals[:, 4:4 + C],
                                op=OP.subtract)
        nc.vector.tensor_scalar(out=vals[:, 4 + C:NB], in0=dsl[:],
                                scalar1=vals[:, 1:2], scalar2=None,
                                op0=OP.mult)

        # ---- broadcast to window partitions ----
        # bcw cols: 0 = n+1, 1 = global window start, 2:2+C = xs,
        # 2+C:2+2C = slope
        NW = 2 + 2 * C
        bcw = psum.tile([P, NW], F32)
        nc.tensor.matmul(bcw[:], lhsT=Bw[:], rhs=vals[:, 2:NB],
                         start=True, stop=True)
        bcs = sm.tile([P, NW], F32)
        nc.scalar.copy(out=bcs[:], in_=bcw[:])
        ixwf = sm.tile([P, 1], F32)
        nc.vector.tensor_tensor(out=ixwf[:], in0=bcs[:, 1:2], in1=wofff[:],
                                op=OP.add)
        ixw = sm.tile([P, 1], I32)
        nc.vector.tensor_copy(out=ixw[:], in_=ixwf[:])
        HF = FPP // 2
        ixw2f = sm.tile([P, 1], F32)
        nc.vector.tensor_scalar(out=ixw2f[:], in0=ixwf[:], scalar1=float(HF),
                                scalar2=None, op0=OP.add)
        ixw2 = sm.tile([P, 1], I32)
        nc.vector.tensor_copy(out=ixw2[:], in_=ixw2f[:])

        # ---- mask + interp, computable before the window data arrives ----
        # valid = (w >= n+1): everything at or beyond frame e keeps x.
        gm = win.tile([P, FPP * C], U8)
        nc.vector.tensor_scalar(
            out=gm[:].rearrange("p (f c) -> p f c", c=C),
            in0=_bcast_inner(wgrid[:], C), scalar1=bcs[:, 0:1], scalar2=None,
            op0=OP.is_ge)
        prods = []
        for h in range(2):
            ph = win.tile([P, HF * C], F32, name=f"prod{h}")
            p3 = ph[:].rearrange("p (f c) -> p f c", c=C)
            wg = wgrid[:, h * HF:(h + 1) * HF]
            nc.vector.tensor_tensor(
                out=p3, in0=_bcast_inner(wg, C),
                in1=_bcast_mid(bcs[:, 2 + C:2 + 2 * C], HF), op=OP.mult)
            nc.vector.tensor_tensor(
                out=p3, in0=p3, in1=_bcast_mid(bcs[:, 2:2 + C], HF),
                op=OP.add)
            prods.append(ph)

        # ---- gather window halves, keep original x where valid, scatter ----
        offs = [ixw, ixw2]
        for h in range(2):
            xwh = win.tile([P, HF * C], F32, name=f"xw{h}")
            nc.gpsimd.indirect_dma_start(
                out=xwh[:], out_offset=None, in_=xv,
                in_offset=bass.IndirectOffsetOnAxis(ap=offs[h][:, 0:1], axis=0))
            nc.vector.copy_predicated(
                out=prods[h][:], mask=gm[:, h * HF * C:(h + 1) * HF * C],
                data=xwh[:])
            sc = nc.gpsimd.indirect_dma_start(
                out=yv,
                out_offset=bass.IndirectOffsetOnAxis(ap=offs[h][:, 0:1], axis=0),
                in_=prods[h][:], in_offset=None)
            for st in stores:
                add_dep_helper(_ins(sc), _ins(st),
                               reason="scatter windows after bulk stores")


_NC = None


def _get_nc():
    global _NC
    if _NC is None:
        nc = bacc.Bacc("TRN2", target_bir_lowering=False, debug=False,
                       num_devices=NCORES)
        x = nc.dram_tensor("x", [R, T, C], F32, kind="ExternalInput")
        y = nc.dram_tensor("y", [R, T, C], F32, kind="ExternalOutput")
        with tile.TileContext(nc) as tc:
            build_kernel(tc, x.ap(), y.ap())
        nc.compile()
        _NC = nc
    return _NC


def kernel(x):
    from concourse.bass_utils import run_bass_kernel_spmd

    x = np.ascontiguousarray(x, dtype=np.float32)
    assert x.shape == (B, T, A, D), x.shape
    xr = x.reshape(NCORES, R, T, C)
    nc = _get_nc()
    in_maps = [{"x": xr[i]} for i in range(NCORES)]
    res = run_bass_kernel_spmd(nc, in_maps, core_ids=list(range(NCORES)))
    out = np.stack([res.results[i]["y"] for i in range(NCORES)])
    return out.reshape(B, T, A, D)
